# revision 12
# baseline (speedup 1.0000x reference)
"""Cross-modal attention TRN2 kernel (bf16 on-chip).

Problem: B=4, N=2048, IN_DIM=DIM=1024, HEADS=8, D_HEAD=128, scale=DIM**-0.5.
  q = x_a @ W_q.T ; k,v = split(x_b @ W_kv.T) ; per-head softmax(q k^T/32) v ;
  out = merge_heads @ W_out.T + b_out

Sharding over 8 cores: core c -> batch b=c//2, head-half hh=c%2 (4 heads,
512 of DIM).  W_q/W_kv column-sharded, W_out row-sharded (Megatron); each
core emits a partial output projection y_cT = (W_out[:, slice] @ O_half)
of shape [DIM, N] in bf16; host sums the two head-half partials per batch
in f32, adds b_out, transposes back.

All on-chip operands are bf16 (PSUM accumulation stays f32):
 - halves HBM traffic (12 MB in / 4 MB out per core),
 - enables the compiler's fast-weight-load path so LDWEIGHTS (~53 ns)
   hides fully under the 512-row matmuls (~216 ns) -- fp32r paid ~204 ns
   per load which stretched the matmul cadence to ~253 ns.

Device layout: everything transposed ([feature, token]) so all matmuls
contract over the partition dim.
  phase 1: Q^T = WqT.T @ xaT, K^T likewise, V (natural [j, dv]), 512-wide
           token blocks, outputs copied to SBUF as bf16.
  phase 2: per (head, 1024-token block): s^T[j,i] on PE; exp on ACT (no
           max subtraction -- |s*scale| < ~1 by construction); PV and a
           ones-row matmul (denominator) accumulate over j-tiles in PSUM.
           Normalize drain: ACT copies the denominator row out of PSUM
           (frees the bank fast), DVE reciprocal_approx_fast (the exact
           reciprocal is ~6.6 cyc/elem and stalled the PE ~5 us per
           block), GpSimd broadcast, DVE multiply.
  phase 3: y^T = WoT.T @ O^T, staged to SBUF bf16, DMA to DRAM.
"""

import numpy as np

B, N, IN_DIM, DIM, HEADS = 4, 2048, 1024, 1024, 8
D_HEAD = DIM // HEADS          # 128
SCALE = DIM ** -0.5            # 1/32
NCORES = 8
HH = HEADS // 2                # 4 heads per core
DVC = HH * D_HEAD              # 512 dv per core
P = 128
KT = IN_DIM // P               # 8 contraction tiles
NJT = N // P                   # 16 j tiles
NIB = N // 512                 # 4 i-blocks of 512 (phase 3)
IB2 = N // 1024                # 2 i-blocks of 1024 (phase 2)

_TRACE = False
REPS = 1
LAST_EXEC_NS = None
LAST_RESULTS = None


def _build_nc(reps=1):
    import concourse.tile as tile
    from concourse import bacc, mybir

    f32 = mybir.dt.float32
    bf16 = mybir.dt.bfloat16
    fp8 = mybir.dt.float8e4
    DR = mybir.MatmulPerfMode.DoubleRow
    Exp = mybir.ActivationFunctionType.Exp

    nc = bacc.Bacc("TRN2", debug=False, num_devices=NCORES)

    # fp8 projections were tried and reverted: e4m3's ~3.6% rms rounding on
    # x and W put the end-to-end error at 1.9e-2 -- at the 2e-2 gate.
    xaT = nc.dram_tensor("xaT", [IN_DIM, N], bf16, kind="ExternalInput").ap()
    xbT = nc.dram_tensor("xbT", [IN_DIM, N], bf16, kind="ExternalInput").ap()
    wqT = nc.dram_tensor("wqT", [IN_DIM, DVC], bf16, kind="ExternalInput").ap()
    wkT = nc.dram_tensor("wkT", [IN_DIM, DVC], bf16, kind="ExternalInput").ap()
    wvT = nc.dram_tensor("wvT", [IN_DIM, DVC], bf16, kind="ExternalInput").ap()
    woT = nc.dram_tensor("woT", [DVC, DIM], bf16, kind="ExternalInput").ap()
    ones_d = nc.dram_tensor("ones", [P, 1], bf16, kind="ExternalInput").ap()
    yT = nc.dram_tensor("yT", [DIM, N], bf16, kind="ExternalOutput").ap()
    SCALE_EXP = SCALE

    with tile.TileContext(nc) as tc:
      for _rep in range(reps):
        with tc.tile_pool(name="persist", bufs=1) as persist:
            qT_sb = persist.tile([P, HH, N], bf16)      # [d%128, head, i]
            kT_sb = persist.tile([P, HH, N], bf16)      # [d%128, head, j]
            v_sb = persist.tile([P, NJT, DVC], bf16)    # [j%128, jt, dv]
            oT_ts = [[persist.tile([P, 1024], bf16, tag=f"o{h}_{bb}",
                                   name=f"o{h}_{bb}")
                      for bb in range(IB2)] for h in range(HH)]
            ones_sb = persist.tile([P, 1], bf16)
            nc.sync.dma_start(out=ones_sb, in_=ones_d)
            wo_sb = persist.tile([P, HH, DIM], bf16)
            # Warm the ACT exp table set during phase 1 (the first table
            # load costs ~2.7us; don't pay it at the phase-2 pipeline fill).
            exp_warm = persist.tile([P, 1], f32)
            nc.scalar.activation(exp_warm, ones_sb, Exp, scale=SCALE_EXP)

            # ---------------- phase 1: projections ----------------
            BW = 512  # streaming block width (1 PSUM bank of f32)
            NB = N // BW
            with tc.tile_pool(name="wpool", bufs=1) as wpool, \
                 tc.tile_pool(name="xblk", bufs=3) as xblk, \
                 tc.tile_pool(name="psum1", bufs=4, space="PSUM") as psum1:
                wq_t = wpool.tile([P, KT, DVC], bf16, tag="wq", name="wq")
                wk_t = wpool.tile([P, KT, DVC], bf16, tag="wk", name="wk")
                wv_t = wpool.tile([P, KT, DVC], bf16, tag="wv", name="wv")

                for ib in range(NB):
                    xa_blk = xblk.tile([P, KT, BW], bf16, tag="xa")
                    nc.sync.dma_start(
                        out=xa_blk,
                        in_=xaT[:, ib * BW:(ib + 1) * BW]
                        .rearrange("(kt p) i -> p kt i", p=P))
                    if ib == 0:
                        # after the first x block so the first matmul's
                        # operands land earliest in DMA queue order
                        nc.sync.dma_start(
                            out=wq_t,
                            in_=wqT.rearrange("(kt p) d -> p kt d", p=P))
                    for dt in range(HH):
                        ps = psum1.tile([P, BW], f32, tag="ps1")
                        for kt in range(KT):
                            nc.tensor.matmul(
                                ps,
                                wq_t[:, kt, dt * P:(dt + 1) * P],
                                xa_blk[:, kt, :],
                                start=(kt == 0), stop=(kt == KT - 1))
                        nc.vector.tensor_copy(
                            qT_sb[:, dt, ib * BW:(ib + 1) * BW], ps)

                for jb in range(NB):
                    xb_blk = xblk.tile([P, KT, BW], bf16, tag="xb")
                    nc.sync.dma_start(
                        out=xb_blk,
                        in_=xbT[:, jb * BW:(jb + 1) * BW]
                        .rearrange("(kt p) i -> p kt i", p=P))
                    if jb == 0:
                        nc.sync.dma_start(
                            out=wk_t,
                            in_=wkT.rearrange("(kt p) d -> p kt d", p=P))
                        nc.sync.dma_start(
                            out=wv_t,
                            in_=wvT.rearrange("(kt p) d -> p kt d", p=P))
                    for dt in range(HH):
                        ps = psum1.tile([P, BW], f32, tag="ps1")
                        for kt in range(KT):
                            nc.tensor.matmul(
                                ps,
                                wk_t[:, kt, dt * P:(dt + 1) * P],
                                xb_blk[:, kt, :],
                                start=(kt == 0), stop=(kt == KT - 1))
                        nc.vector.tensor_copy(
                            kT_sb[:, dt, jb * BW:(jb + 1) * BW], ps)
                    for j2 in range(BW // P):
                        jt = jb * (BW // P) + j2
                        ps = psum1.tile([P, DVC], f32, tag="psv")
                        for kt in range(KT):
                            nc.tensor.matmul(
                                ps,
                                xb_blk[:, kt, j2 * P:(j2 + 1) * P],
                                wv_t[:, kt, :],
                                start=(kt == 0), stop=(kt == KT - 1))
                        nc.vector.tensor_copy(v_sb[:, jt, :], ps)

            # ---------------- phase 2: attention ----------------
            with tc.tile_pool(name="expp", bufs=6) as expp, \
                 tc.tile_pool(name="sump", bufs=2) as sump, \
                 tc.tile_pool(name="drainp", bufs=2) as drainp, \
                 tc.tile_pool(name="dotsp", bufs=2, space="PSUM") as dotsp, \
                 tc.tile_pool(name="avp", bufs=1, space="PSUM") as avp, \
                 tc.tile_pool(name="denp", bufs=1, space="PSUM") as denp:
                LAG = 2   # PV/ones trail dots/exp by 2 j-tiles so the PE
                          # never waits on the ACT exp of the current tile
                # prefetch the output-projection weights; DMA is idle here
                nc.sync.dma_start(
                    out=wo_sb, in_=woT.rearrange("(dt p) e -> p dt e", p=P))
                # Denominator: j-tiles 0..11 are pre-summed in quads on the
                # DVE (3 adds) so one ones-matmul covers 4 tiles; the last 4
                # j-tiles go through per-tile ones-matmuls so the block tail
                # doesn't serialize behind the DVE adds.  bf16 quad-sums add
                # ~0.15% rms to the denominator -- well inside tolerance.
                for ib in range(IB2):
                    for h in range(HH):
                        i0 = ib * 1024
                        po = avp.tile([P, 1024], f32)
                        pd = denp.tile([1, 1024], f32)
                        ets = {}
                        etqs = {}
                        for jt in range(NJT + LAG):
                            if jt < NJT:
                                ps = dotsp.tile([P, 1024], f32, tag="ps")
                                k_l = kT_sb[:, h, jt * P:(jt + 1) * P]
                                for hf in range(2):
                                    nc.tensor.matmul(
                                        ps[:, hf * 512:(hf + 1) * 512],
                                        k_l,
                                        qT_sb[:, h,
                                              i0 + hf * 512:i0 + (hf + 1) * 512],
                                        start=True, stop=True)
                                et = expp.tile([P, 1024], bf16, tag="exp")
                                nc.scalar.activation(et, ps, Exp,
                                                     scale=SCALE_EXP)
                                ets[jt] = et
                            if jt >= LAG:
                                jd = jt - LAG
                                et = ets[jd]
                                v_l = v_sb[:, jd, h * P:(h + 1) * P]
                                for hf in range(2):
                                    sl = slice(hf * 512, (hf + 1) * 512)
                                    nc.tensor.matmul(
                                        po[:, sl], v_l, et[:, sl],
                                        start=(jd == 0), stop=(jd == NJT - 1))
                                if jd >= 12:
                                    for hf in range(2):
                                        sl = slice(hf * 512, (hf + 1) * 512)
                                        nc.tensor.matmul(
                                            pd[:, sl], ones_sb, et[:, sl],
                                            start=False, stop=(jd == NJT - 1))
                            if jt in (3, 7, 11):
                                q = jt // 4
                                s01 = sump.tile([P, 1024], bf16, tag="s01")
                                nc.vector.tensor_add(
                                    s01, ets[4 * q], ets[4 * q + 1])
                                s23 = sump.tile([P, 1024], bf16, tag="s23")
                                nc.vector.tensor_add(
                                    s23, ets[4 * q + 2], ets[4 * q + 3])
                                etq = sump.tile([P, 1024], bf16, tag="etq")
                                nc.vector.tensor_add(etq, s01, s23)
                                etqs[q] = etq
                            if jt in (6, 10, 14):
                                q = (jt - 6) // 4
                                etq = etqs.pop(q)
                                for hf in range(2):
                                    sl = slice(hf * 512, (hf + 1) * 512)
                                    nc.tensor.matmul(
                                        pd[:, sl], ones_sb, etq[:, sl],
                                        start=(q == 0), stop=False)
                        # Drain: free the PV and denominator PSUM banks as
                        # fast as possible (next block's accumulations wait
                        # on them), then normalize off the critical path.
                        osl = oT_ts[h][ib]
                        nc.vector.tensor_copy(osl, po)
                        dsb = drainp.tile([1, 1024], f32, tag="den")
                        nc.scalar.copy(dsb, pd)
                        rcf = drainp.tile([1, 1024], f32, tag="rcf")
                        nc.vector.reciprocal_approx_fast(rcf, dsb)
                        rcb = drainp.tile([1, 1024], bf16, tag="rcb")
                        nc.vector.tensor_copy(rcb, rcf)
                        bc = drainp.tile([P, 1024], bf16, tag="bc")
                        nc.gpsimd.partition_broadcast(bc, rcb)
                        nc.vector.tensor_mul(osl, osl, bc)

                # ---------------- phase 3: output projection ----------------
                # y-psum tiles share the dots pool slots (tag "ps"), which
                # free as the exp of the final j-tiles completes -- a fresh
                # PSUM pool would wait on the whole attention stack instead.
                with tc.tile_pool(name="ystage", bufs=4) as ystage:
                    for ib in range(NIB):
                        bb, half = divmod(ib, 2)
                        for e8 in range(DIM // P):
                            ps = dotsp.tile([P, 512], f32, tag="ps")
                            for dt in range(HH):
                                nc.tensor.matmul(
                                    ps,
                                    wo_sb[:, dt, e8 * P:(e8 + 1) * P],
                                    oT_ts[dt][bb][:, half * 512:(half + 1) * 512],
                                    start=(dt == 0), stop=(dt == HH - 1))
                            ys = ystage.tile([P, 512], bf16, tag="ys")
                            if ib % 2 == 0:
                                nc.vector.tensor_copy(ys, ps)
                            else:
                                nc.scalar.copy(ys, ps)
                            nc.sync.dma_start(
                                out=yT[e8 * P:(e8 + 1) * P,
                                       ib * 512:(ib + 1) * 512],
                                in_=ys)

    nc.compile()
    return nc


_nc_by_reps = {}


def _get_nc(reps=1):
    if reps not in _nc_by_reps:
        _nc_by_reps[reps] = _build_nc(reps)
    return _nc_by_reps[reps]


def _make_in_maps(x_a, x_b, W_q, W_kv, W_out):
    from concourse import mybir
    BF = mybir.dt.np(mybir.dt.bfloat16)
    xaT = [np.ascontiguousarray(x_a[b].T).astype(BF) for b in range(B)]
    xbT = [np.ascontiguousarray(x_b[b].T).astype(BF) for b in range(B)]
    in_maps = []
    for c in range(NCORES):
        b, hh = divmod(c, 2)
        hs = hh * DVC
        in_maps.append({
            "xaT": xaT[b],
            "xbT": xbT[b],
            "wqT": np.ascontiguousarray(W_q[hs:hs + DVC].T).astype(BF),
            "wkT": np.ascontiguousarray(W_kv[hs:hs + DVC].T).astype(BF),
            "wvT": np.ascontiguousarray(
                W_kv[DIM + hs:DIM + hs + DVC].T).astype(BF),
            "woT": np.ascontiguousarray(W_out[:, hs:hs + DVC].T).astype(BF),
            "ones": np.ones((P, 1), dtype=BF),
        })
    return in_maps


def kernel(x_a, x_b, W_q, W_kv, W_out, b_out):
    global LAST_EXEC_NS, LAST_RESULTS
    from concourse import bass_utils

    x_a = np.asarray(x_a, dtype=np.float32)
    x_b = np.asarray(x_b, dtype=np.float32)
    W_q = np.asarray(W_q, dtype=np.float32)
    W_kv = np.asarray(W_kv, dtype=np.float32)
    W_out = np.asarray(W_out, dtype=np.float32)
    b_out = np.asarray(b_out, dtype=np.float32)

    nc = _get_nc(REPS)
    in_maps = _make_in_maps(x_a, x_b, W_q, W_kv, W_out)

    res = bass_utils.run_bass_kernel_spmd(
        nc, in_maps, core_ids=list(range(NCORES)), trace=_TRACE)
    LAST_EXEC_NS = res.exec_time_ns
    LAST_RESULTS = res

    out = np.empty((B, N, DIM), dtype=np.float32)
    for b in range(B):
        acc = (res.results[2 * b]["yT"].astype(np.float32)
               + res.results[2 * b + 1]["yT"].astype(np.float32))
        out[b] = acc.T + b_out
    return out


def bench(inputs, reps_pair=(1, 9), iters=5):
    """Measure on-device time per kernel body via rep-delta wall timing."""
    import time
    from concourse import bass_utils
    ins = {k: np.asarray(v, dtype=np.float32) for k, v in inputs.items()
           if k != "b_out"}
    in_maps = _make_in_maps(ins["x_a"], ins["x_b"], ins["W_q"], ins["W_kv"],
                            ins["W_out"])
    walls = {}
    for reps in reps_pair:
        nc = _get_nc(reps)
        # warm-up (compile+cache)
        bass_utils.run_bass_kernel_spmd(nc, in_maps, core_ids=list(range(NCORES)))
        ts = []
        for _ in range(iters):
            t0 = time.perf_counter()
            bass_utils.run_bass_kernel_spmd(nc, in_maps,
                                            core_ids=list(range(NCORES)))
            ts.append(time.perf_counter() - t0)
        walls[reps] = min(ts)
        print(f"reps={reps}: wall min={walls[reps]*1e3:.2f} ms  all={[f'{t*1e3:.1f}' for t in ts]}")
    r0, r1 = reps_pair
    ns = (walls[r1] - walls[r0]) / (r1 - r0) * 1e9
    print(f"per-body device time: {ns:.0f} ns")
    return ns


# revision 15
# speedup vs baseline: 1.0271x; 1.0271x over previous
"""Cross-modal attention TRN2 kernel (bf16 on-chip).

Problem: B=4, N=2048, IN_DIM=DIM=1024, HEADS=8, D_HEAD=128, scale=DIM**-0.5.
  q = x_a @ W_q.T ; k,v = split(x_b @ W_kv.T) ; per-head softmax(q k^T/32) v ;
  out = merge_heads @ W_out.T + b_out

Sharding over 8 cores: core c -> batch b=c//2, head-half hh=c%2 (4 heads,
512 of DIM).  W_q/W_kv column-sharded, W_out row-sharded (Megatron); each
core emits a partial output projection y_cT = (W_out[:, slice] @ O_half)
of shape [DIM, N] in bf16; host sums the two head-half partials per batch
in f32, adds b_out, transposes back.

All on-chip operands are bf16 (PSUM accumulation stays f32):
 - halves HBM traffic (12 MB in / 4 MB out per core),
 - enables the compiler's fast-weight-load path so LDWEIGHTS (~53 ns)
   hides fully under the 512-row matmuls (~216 ns) -- fp32r paid ~204 ns
   per load which stretched the matmul cadence to ~253 ns.

Device layout: everything transposed ([feature, token]) so all matmuls
contract over the partition dim.
  phase 1: Q^T = WqT.T @ xaT, K^T likewise, V (natural [j, dv]), 512-wide
           token blocks, outputs copied to SBUF as bf16.
  phase 2: per (head, 1024-token block): s^T[j,i] on PE; exp on ACT (no
           max subtraction -- |s*scale| < ~1 by construction); PV and a
           ones-row matmul (denominator) accumulate over j-tiles in PSUM.
           Normalize drain: ACT copies the denominator row out of PSUM
           (frees the bank fast), DVE reciprocal_approx_fast (the exact
           reciprocal is ~6.6 cyc/elem and stalled the PE ~5 us per
           block), GpSimd broadcast, DVE multiply.
  phase 3: y^T = WoT.T @ O^T, staged to SBUF bf16, DMA to DRAM.
"""

import numpy as np

B, N, IN_DIM, DIM, HEADS = 4, 2048, 1024, 1024, 8
D_HEAD = DIM // HEADS          # 128
SCALE = DIM ** -0.5            # 1/32
NCORES = 8
HH = HEADS // 2                # 4 heads per core
DVC = HH * D_HEAD              # 512 dv per core
P = 128
KT = IN_DIM // P               # 8 contraction tiles
NJT = N // P                   # 16 j tiles
NIB = N // 512                 # 4 i-blocks of 512 (phase 3)
IB2 = N // 1024                # 2 i-blocks of 1024 (phase 2)

_TRACE = False
REPS = 1
LAST_EXEC_NS = None
LAST_RESULTS = None


def _build_nc(reps=1):
    import concourse.tile as tile
    from concourse import bacc, mybir

    f32 = mybir.dt.float32
    bf16 = mybir.dt.bfloat16
    fp8 = mybir.dt.float8e4
    DR = mybir.MatmulPerfMode.DoubleRow
    Exp = mybir.ActivationFunctionType.Exp

    nc = bacc.Bacc("TRN2", debug=False, num_devices=NCORES)

    # fp8 projections were tried and reverted: e4m3's ~3.6% rms rounding on
    # x and W put the end-to-end error at 1.9e-2 -- at the 2e-2 gate.
    xaT = nc.dram_tensor("xaT", [IN_DIM, N], bf16, kind="ExternalInput").ap()
    xbT = nc.dram_tensor("xbT", [IN_DIM, N], bf16, kind="ExternalInput").ap()
    wqT = nc.dram_tensor("wqT", [IN_DIM, DVC], bf16, kind="ExternalInput").ap()
    wkT = nc.dram_tensor("wkT", [IN_DIM, DVC], bf16, kind="ExternalInput").ap()
    wvT = nc.dram_tensor("wvT", [IN_DIM, DVC], bf16, kind="ExternalInput").ap()
    woT = nc.dram_tensor("woT", [DVC, DIM], bf16, kind="ExternalInput").ap()
    ones_d = nc.dram_tensor("ones", [P, 1], bf16, kind="ExternalInput").ap()
    yT = nc.dram_tensor("yT", [DIM, N], bf16, kind="ExternalOutput").ap()
    SCALE_EXP = SCALE

    with tile.TileContext(nc) as tc:
      for _rep in range(reps):
        with tc.tile_pool(name="persist", bufs=1) as persist:
            qT_sb = persist.tile([P, HH, N], bf16)      # [d%128, head, i]
            kT_sb = persist.tile([P, HH, N], bf16)      # [d%128, head, j]
            v_sb = persist.tile([P, NJT, DVC], bf16)    # [j%128, jt, dv]
            oT_ts = [[persist.tile([P, 1024], bf16, tag=f"o{h}_{bb}",
                                   name=f"o{h}_{bb}")
                      for bb in range(IB2)] for h in range(HH)]
            ones_sb = persist.tile([P, 1], bf16)
            nc.sync.dma_start(out=ones_sb, in_=ones_d)
            wo_sb = persist.tile([P, HH, DIM], bf16)
            # Warm the ACT exp table set during phase 1 (the first table
            # load costs ~2.7us; don't pay it at the phase-2 pipeline fill).
            exp_warm = persist.tile([P, 1], f32)
            nc.scalar.activation(exp_warm, ones_sb, Exp, scale=SCALE_EXP)

            # ---------------- phase 1: projections ----------------
            BW = 512  # streaming block width (1 PSUM bank of f32)
            NB = N // BW
            with tc.tile_pool(name="wpool", bufs=1) as wpool, \
                 tc.tile_pool(name="xblk", bufs=3) as xblk, \
                 tc.tile_pool(name="psum1", bufs=4, space="PSUM") as psum1:
                wq_t = wpool.tile([P, KT, DVC], bf16, tag="wq", name="wq")
                wk_t = wpool.tile([P, KT, DVC], bf16, tag="wk", name="wk")
                wv_t = wpool.tile([P, KT, DVC], bf16, tag="wv", name="wv")

                for ib in range(NB):
                    xa_blk = xblk.tile([P, KT, BW], bf16, tag="xa")
                    nc.sync.dma_start(
                        out=xa_blk,
                        in_=xaT[:, ib * BW:(ib + 1) * BW]
                        .rearrange("(kt p) i -> p kt i", p=P))
                    if ib == 0:
                        # after the first x block so the first matmul's
                        # operands land earliest in DMA queue order
                        nc.sync.dma_start(
                            out=wq_t,
                            in_=wqT.rearrange("(kt p) d -> p kt d", p=P))
                    for dt in range(HH):
                        ps = psum1.tile([P, BW], f32, tag="ps1")
                        for kt in range(KT):
                            nc.tensor.matmul(
                                ps,
                                wq_t[:, kt, dt * P:(dt + 1) * P],
                                xa_blk[:, kt, :],
                                start=(kt == 0), stop=(kt == KT - 1))
                        nc.vector.tensor_copy(
                            qT_sb[:, dt, ib * BW:(ib + 1) * BW], ps)

                for jb in range(NB):
                    xb_blk = xblk.tile([P, KT, BW], bf16, tag="xb")
                    nc.sync.dma_start(
                        out=xb_blk,
                        in_=xbT[:, jb * BW:(jb + 1) * BW]
                        .rearrange("(kt p) i -> p kt i", p=P))
                    if jb == 0:
                        nc.sync.dma_start(
                            out=wk_t,
                            in_=wkT.rearrange("(kt p) d -> p kt d", p=P))
                        nc.sync.dma_start(
                            out=wv_t,
                            in_=wvT.rearrange("(kt p) d -> p kt d", p=P))
                    for dt in range(HH):
                        ps = psum1.tile([P, BW], f32, tag="ps1")
                        for kt in range(KT):
                            nc.tensor.matmul(
                                ps,
                                wk_t[:, kt, dt * P:(dt + 1) * P],
                                xb_blk[:, kt, :],
                                start=(kt == 0), stop=(kt == KT - 1))
                        nc.vector.tensor_copy(
                            kT_sb[:, dt, jb * BW:(jb + 1) * BW], ps)
                    for j2 in range(BW // P):
                        jt = jb * (BW // P) + j2
                        ps = psum1.tile([P, DVC], f32, tag="psv")
                        for kt in range(KT):
                            nc.tensor.matmul(
                                ps,
                                xb_blk[:, kt, j2 * P:(j2 + 1) * P],
                                wv_t[:, kt, :],
                                start=(kt == 0), stop=(kt == KT - 1))
                        nc.vector.tensor_copy(v_sb[:, jt, :], ps)

            # ---------------- phase 2: attention ----------------
            with tc.tile_pool(name="expp", bufs=6) as expp, \
                 tc.tile_pool(name="sump", bufs=2) as sump, \
                 tc.tile_pool(name="drainp", bufs=2) as drainp, \
                 tc.tile_pool(name="dotsp", bufs=2, space="PSUM") as dotsp, \
                 tc.tile_pool(name="avp", bufs=1, space="PSUM") as avp, \
                 tc.tile_pool(name="denp", bufs=1, space="PSUM") as denp:
                LAG = 2   # PV/ones trail dots/exp by 2 j-tiles so the PE
                          # never waits on the ACT exp of the current tile
                # prefetch the output-projection weights; DMA is idle here
                nc.sync.dma_start(
                    out=wo_sb, in_=woT.rearrange("(dt p) e -> p dt e", p=P))
                # Denominator: j-tiles 0..11 are pre-summed in quads on the
                # DVE (3 adds each) so one ones-matmul covers 4 tiles; tiles
                # 12..15 are pre-summed in pairs (1 add) so the block tail
                # doesn't serialize behind a deep add tree.  The quad
                # ones-matmuls are scheduled at jt 8/11/14 -- late enough
                # that the previous block's drain chain (po-copy, pd-copy,
                # reciprocal, broadcast, multiply) and this block's adds have
                # cleared the DVE queue before the PE reaches them.  bf16
                # partial sums add ~0.15% rms to the denominator.
                for ib in range(IB2):
                    for h in range(HH):
                        i0 = ib * 1024
                        po = avp.tile([P, 1024], f32)
                        pd = denp.tile([1, 1024], f32)
                        ets = {}
                        etqs = {}
                        for jt in range(NJT + LAG):
                            if jt < NJT:
                                ps = dotsp.tile([P, 1024], f32, tag="ps")
                                k_l = kT_sb[:, h, jt * P:(jt + 1) * P]
                                for hf in range(2):
                                    nc.tensor.matmul(
                                        ps[:, hf * 512:(hf + 1) * 512],
                                        k_l,
                                        qT_sb[:, h,
                                              i0 + hf * 512:i0 + (hf + 1) * 512],
                                        start=True, stop=True)
                                et = expp.tile([P, 1024], bf16, tag="exp")
                                nc.scalar.activation(et, ps, Exp,
                                                     scale=SCALE_EXP)
                                ets[jt] = et
                            if jt >= LAG:
                                jd = jt - LAG
                                et = ets[jd]
                                v_l = v_sb[:, jd, h * P:(h + 1) * P]
                                for hf in range(2):
                                    sl = slice(hf * 512, (hf + 1) * 512)
                                    nc.tensor.matmul(
                                        po[:, sl], v_l, et[:, sl],
                                        start=(jd == 0), stop=(jd == NJT - 1))
                            if jt in (3, 7, 11):
                                q = jt // 4
                                s01 = sump.tile([P, 1024], bf16, tag="s01")
                                nc.vector.tensor_add(
                                    s01, ets[4 * q], ets[4 * q + 1])
                                s23 = sump.tile([P, 1024], bf16, tag="s23")
                                nc.vector.tensor_add(
                                    s23, ets[4 * q + 2], ets[4 * q + 3])
                                etq = sump.tile([P, 1024], bf16, tag="etq")
                                nc.vector.tensor_add(etq, s01, s23)
                                etqs[q] = etq
                            if jt in (15, 17):
                                t0 = 12 if jt == 15 else 14
                                ep = sump.tile([P, 1024], bf16, tag="ep")
                                nc.vector.tensor_add(
                                    ep, ets[t0], ets[t0 + 1])
                                etqs[("p", t0)] = ep
                            if jt in (8, 11, 14):
                                q = {8: 0, 11: 1, 14: 2}[jt]
                                etq = etqs.pop(q)
                                for hf in range(2):
                                    sl = slice(hf * 512, (hf + 1) * 512)
                                    nc.tensor.matmul(
                                        pd[:, sl], ones_sb, etq[:, sl],
                                        start=(q == 0), stop=False)
                            if jt in (16, 17):
                                t0 = 12 if jt == 16 else 14
                                ep = etqs.pop(("p", t0))
                                for hf in range(2):
                                    sl = slice(hf * 512, (hf + 1) * 512)
                                    nc.tensor.matmul(
                                        pd[:, sl], ones_sb, ep[:, sl],
                                        start=False, stop=(t0 == 14))
                        # Drain: free the PV and denominator PSUM banks as
                        # fast as possible (next block's accumulations wait
                        # on them), then normalize off the critical path.
                        osl = oT_ts[h][ib]
                        nc.vector.tensor_copy(osl, po)
                        dsb = drainp.tile([1, 1024], f32, tag="den")
                        # on DVE, not ACT: the ACT queue must flow straight
                        # from this block's exp(15) into the next block's
                        # exp(0) or the PE stalls on the dots pipeline refill
                        nc.vector.tensor_copy(dsb, pd)
                        rcf = drainp.tile([1, 1024], f32, tag="rcf")
                        nc.vector.reciprocal_approx_fast(rcf, dsb)
                        rcb = drainp.tile([1, 1024], bf16, tag="rcb")
                        nc.vector.tensor_copy(rcb, rcf)
                        bc = drainp.tile([P, 1024], bf16, tag="bc")
                        nc.gpsimd.partition_broadcast(bc, rcb)
                        nc.vector.tensor_mul(osl, osl, bc)

                # ---------------- phase 3: output projection ----------------
                # y-psum tiles share the dots pool slots (tag "ps"), which
                # free as the exp of the final j-tiles completes -- a fresh
                # PSUM pool would wait on the whole attention stack instead.
                with tc.tile_pool(name="ystage", bufs=4) as ystage:
                    for ib in range(NIB):
                        bb, half = divmod(ib, 2)
                        for e8 in range(DIM // P):
                            ps = dotsp.tile([P, 512], f32, tag="ps")
                            for dt in range(HH):
                                nc.tensor.matmul(
                                    ps,
                                    wo_sb[:, dt, e8 * P:(e8 + 1) * P],
                                    oT_ts[dt][bb][:, half * 512:(half + 1) * 512],
                                    start=(dt == 0), stop=(dt == HH - 1))
                            ys = ystage.tile([P, 512], bf16, tag="ys")
                            if ib % 2 == 0:
                                nc.vector.tensor_copy(ys, ps)
                            else:
                                nc.scalar.copy(ys, ps)
                            nc.sync.dma_start(
                                out=yT[e8 * P:(e8 + 1) * P,
                                       ib * 512:(ib + 1) * 512],
                                in_=ys)

    nc.compile()
    return nc


_nc_by_reps = {}


def _get_nc(reps=1):
    if reps not in _nc_by_reps:
        _nc_by_reps[reps] = _build_nc(reps)
    return _nc_by_reps[reps]


def _make_in_maps(x_a, x_b, W_q, W_kv, W_out):
    from concourse import mybir
    BF = mybir.dt.np(mybir.dt.bfloat16)
    xaT = [np.ascontiguousarray(x_a[b].T).astype(BF) for b in range(B)]
    xbT = [np.ascontiguousarray(x_b[b].T).astype(BF) for b in range(B)]
    in_maps = []
    for c in range(NCORES):
        b, hh = divmod(c, 2)
        hs = hh * DVC
        in_maps.append({
            "xaT": xaT[b],
            "xbT": xbT[b],
            "wqT": np.ascontiguousarray(W_q[hs:hs + DVC].T).astype(BF),
            "wkT": np.ascontiguousarray(W_kv[hs:hs + DVC].T).astype(BF),
            "wvT": np.ascontiguousarray(
                W_kv[DIM + hs:DIM + hs + DVC].T).astype(BF),
            "woT": np.ascontiguousarray(W_out[:, hs:hs + DVC].T).astype(BF),
            "ones": np.ones((P, 1), dtype=BF),
        })
    return in_maps


def kernel(x_a, x_b, W_q, W_kv, W_out, b_out):
    global LAST_EXEC_NS, LAST_RESULTS
    from concourse import bass_utils

    x_a = np.asarray(x_a, dtype=np.float32)
    x_b = np.asarray(x_b, dtype=np.float32)
    W_q = np.asarray(W_q, dtype=np.float32)
    W_kv = np.asarray(W_kv, dtype=np.float32)
    W_out = np.asarray(W_out, dtype=np.float32)
    b_out = np.asarray(b_out, dtype=np.float32)

    nc = _get_nc(REPS)
    in_maps = _make_in_maps(x_a, x_b, W_q, W_kv, W_out)

    res = bass_utils.run_bass_kernel_spmd(
        nc, in_maps, core_ids=list(range(NCORES)), trace=_TRACE)
    LAST_EXEC_NS = res.exec_time_ns
    LAST_RESULTS = res

    out = np.empty((B, N, DIM), dtype=np.float32)
    for b in range(B):
        acc = (res.results[2 * b]["yT"].astype(np.float32)
               + res.results[2 * b + 1]["yT"].astype(np.float32))
        out[b] = acc.T + b_out
    return out


def bench(inputs, reps_pair=(1, 9), iters=5):
    """Measure on-device time per kernel body via rep-delta wall timing."""
    import time
    from concourse import bass_utils
    ins = {k: np.asarray(v, dtype=np.float32) for k, v in inputs.items()
           if k != "b_out"}
    in_maps = _make_in_maps(ins["x_a"], ins["x_b"], ins["W_q"], ins["W_kv"],
                            ins["W_out"])
    walls = {}
    for reps in reps_pair:
        nc = _get_nc(reps)
        # warm-up (compile+cache)
        bass_utils.run_bass_kernel_spmd(nc, in_maps, core_ids=list(range(NCORES)))
        ts = []
        for _ in range(iters):
            t0 = time.perf_counter()
            bass_utils.run_bass_kernel_spmd(nc, in_maps,
                                            core_ids=list(range(NCORES)))
            ts.append(time.perf_counter() - t0)
        walls[reps] = min(ts)
        print(f"reps={reps}: wall min={walls[reps]*1e3:.2f} ms  all={[f'{t*1e3:.1f}' for t in ts]}")
    r0, r1 = reps_pair
    ns = (walls[r1] - walls[r0]) / (r1 - r0) * 1e9
    print(f"per-body device time: {ns:.0f} ns")
    return ns


# revision 17
# speedup vs baseline: 1.0621x; 1.0340x over previous
"""Cross-modal attention TRN2 kernel (bf16 on-chip).

Problem: B=4, N=2048, IN_DIM=DIM=1024, HEADS=8, D_HEAD=128, scale=DIM**-0.5.
  q = x_a @ W_q.T ; k,v = split(x_b @ W_kv.T) ; per-head softmax(q k^T/32) v ;
  out = merge_heads @ W_out.T + b_out

Sharding over 8 cores: core c -> batch b=c//2, head-half hh=c%2 (4 heads,
512 of DIM).  W_q/W_kv column-sharded, W_out row-sharded (Megatron); each
core emits a partial output projection y_cT = (W_out[:, slice] @ O_half)
of shape [DIM, N] in bf16; host sums the two head-half partials per batch
in f32, adds b_out, transposes back.

All on-chip operands are bf16 (PSUM accumulation stays f32):
 - halves HBM traffic (12 MB in / 4 MB out per core),
 - enables the compiler's fast-weight-load path so LDWEIGHTS (~53 ns)
   hides fully under the 512-row matmuls (~216 ns) -- fp32r paid ~204 ns
   per load which stretched the matmul cadence to ~253 ns.

Device layout: everything transposed ([feature, token]) so all matmuls
contract over the partition dim.
  phase 1: Q^T = WqT.T @ xaT, K^T likewise, V (natural [j, dv]), 512-wide
           token blocks, outputs copied to SBUF as bf16.
  phase 2: per (head, 1024-token block): s^T[j,i] on PE; exp on ACT (no
           max subtraction -- |s*scale| < ~1 by construction); PV and a
           ones-row matmul (denominator) accumulate over j-tiles in PSUM.
           Normalize drain: ACT copies the denominator row out of PSUM
           (frees the bank fast), DVE reciprocal_approx_fast (the exact
           reciprocal is ~6.6 cyc/elem and stalled the PE ~5 us per
           block), GpSimd broadcast, DVE multiply.
  phase 3: y^T = WoT.T @ O^T, staged to SBUF bf16, DMA to DRAM.
"""

import numpy as np

B, N, IN_DIM, DIM, HEADS = 4, 2048, 1024, 1024, 8
D_HEAD = DIM // HEADS          # 128
SCALE = DIM ** -0.5            # 1/32
NCORES = 8
HH = HEADS // 2                # 4 heads per core
DVC = HH * D_HEAD              # 512 dv per core
P = 128
KT = IN_DIM // P               # 8 contraction tiles
NJT = N // P                   # 16 j tiles
NIB = N // 512                 # 4 i-blocks of 512 (phase 3)
IB2 = N // 1024                # 2 i-blocks of 1024 (phase 2)

_TRACE = False
REPS = 1
LAST_EXEC_NS = None
LAST_RESULTS = None


def _build_nc(reps=1):
    import concourse.tile as tile
    from concourse import bacc, mybir

    f32 = mybir.dt.float32
    bf16 = mybir.dt.bfloat16
    fp8 = mybir.dt.float8e4
    DR = mybir.MatmulPerfMode.DoubleRow
    Exp = mybir.ActivationFunctionType.Exp

    nc = bacc.Bacc("TRN2", debug=False, num_devices=NCORES)

    # fp8 projections were tried and reverted: e4m3's ~3.6% rms rounding on
    # x and W put the end-to-end error at 1.9e-2 -- at the 2e-2 gate.
    xaT = nc.dram_tensor("xaT", [IN_DIM, N], bf16, kind="ExternalInput").ap()
    xbT = nc.dram_tensor("xbT", [IN_DIM, N], bf16, kind="ExternalInput").ap()
    wqT = nc.dram_tensor("wqT", [IN_DIM, DVC], bf16, kind="ExternalInput").ap()
    wkT = nc.dram_tensor("wkT", [IN_DIM, DVC], bf16, kind="ExternalInput").ap()
    wvT = nc.dram_tensor("wvT", [IN_DIM, DVC], bf16, kind="ExternalInput").ap()
    woT = nc.dram_tensor("woT", [DVC, DIM], bf16, kind="ExternalInput").ap()
    ones_d = nc.dram_tensor("ones", [P, 1], bf16, kind="ExternalInput").ap()
    yT = nc.dram_tensor("yT", [DIM, N], bf16, kind="ExternalOutput").ap()
    SCALE_EXP = SCALE

    with tile.TileContext(nc) as tc:
      for _rep in range(reps):
        with tc.tile_pool(name="persist", bufs=1) as persist:
            qT_sb = persist.tile([P, HH, N], bf16)      # [d%128, head, i]
            kT_sb = persist.tile([P, HH, N], bf16)      # [d%128, head, j]
            v_sb = persist.tile([P, NJT, DVC], bf16)    # [j%128, jt, dv]
            oT_ts = [[persist.tile([P, 1024], bf16, tag=f"o{h}_{bb}",
                                   name=f"o{h}_{bb}")
                      for bb in range(IB2)] for h in range(HH)]
            ones_sb = persist.tile([P, 1], bf16)
            nc.sync.dma_start(out=ones_sb, in_=ones_d)
            wo_sb = persist.tile([P, HH, DIM], bf16)
            # Warm the ACT exp table set and the GpSimd broadcast library
            # during phase 1 (the first table load costs ~2.7us, the first
            # partition_broadcast pays a library reload; don't pay either at
            # a phase-2 block boundary).
            exp_warm = persist.tile([P, 1], f32)
            nc.scalar.activation(exp_warm, ones_sb, Exp, scale=SCALE_EXP)
            bc_warm = persist.tile([P, 1], bf16)
            nc.gpsimd.partition_broadcast(bc_warm, ones_sb[0:1, :])

            # ---------------- phase 1: projections ----------------
            BW = 512  # streaming block width (1 PSUM bank of f32)
            NB = N // BW
            with tc.tile_pool(name="wpool", bufs=1) as wpool, \
                 tc.tile_pool(name="xblk", bufs=3) as xblk, \
                 tc.tile_pool(name="psum1", bufs=4, space="PSUM") as psum1:
                wq_t = wpool.tile([P, KT, DVC], bf16, tag="wq", name="wq")
                wk_t = wpool.tile([P, KT, DVC], bf16, tag="wk", name="wk")
                wv_t = wpool.tile([P, KT, DVC], bf16, tag="wv", name="wv")

                def dma_x(dst, src, o0, bw):
                    # two dma_starts per x block: round-robin queue
                    # assignment is per-start, so splitting spreads the
                    # transfer across more DMA queues
                    kh = KT // 2
                    for h2 in range(2):
                        nc.sync.dma_start(
                            out=dst[:, h2 * kh:(h2 + 1) * kh, :],
                            in_=src[h2 * (IN_DIM // 2):(h2 + 1) * (IN_DIM // 2),
                                    o0:o0 + bw]
                            .rearrange("(kt p) i -> p kt i", p=P))

                def dma_w(dst, src):
                    # per dt-pair: the first matmuls need only the first
                    # 256 weight columns, so they start ~6us earlier
                    for h2 in range(2):
                        nc.sync.dma_start(
                            out=dst[:, :, h2 * 256:(h2 + 1) * 256],
                            in_=src[:, h2 * 256:(h2 + 1) * 256]
                            .rearrange("(kt p) d -> p kt d", p=P))

                # first block split in two 256-wide halves so the first
                # matmul's 0.5 MB of operands lands as early as possible
                qblocks = [(0, 256), (256, 256)] + \
                          [(BW * i, BW) for i in range(1, NB)]
                for bi, (o0, bw) in enumerate(qblocks):
                    xa_blk = xblk.tile([P, KT, bw], bf16, tag="xa")
                    dma_x(xa_blk, xaT, o0, bw)
                    if bi == 0:
                        dma_w(wq_t, wqT)
                    for dt in range(HH):
                        ps = psum1.tile([P, bw], f32, tag="ps1")
                        for kt in range(KT):
                            nc.tensor.matmul(
                                ps,
                                wq_t[:, kt, dt * P:(dt + 1) * P],
                                xa_blk[:, kt, :],
                                start=(kt == 0), stop=(kt == KT - 1))
                        nc.vector.tensor_copy(
                            qT_sb[:, dt, o0:o0 + bw], ps)

                for jb in range(NB):
                    xb_blk = xblk.tile([P, KT, BW], bf16, tag="xb")
                    dma_x(xb_blk, xbT, jb * BW, BW)
                    if jb == 0:
                        dma_w(wk_t, wkT)
                        dma_w(wv_t, wvT)
                    for dt in range(HH):
                        ps = psum1.tile([P, BW], f32, tag="ps1")
                        for kt in range(KT):
                            nc.tensor.matmul(
                                ps,
                                wk_t[:, kt, dt * P:(dt + 1) * P],
                                xb_blk[:, kt, :],
                                start=(kt == 0), stop=(kt == KT - 1))
                        nc.vector.tensor_copy(
                            kT_sb[:, dt, jb * BW:(jb + 1) * BW], ps)
                    for j2 in range(BW // P):
                        jt = jb * (BW // P) + j2
                        ps = psum1.tile([P, DVC], f32, tag="psv")
                        for kt in range(KT):
                            nc.tensor.matmul(
                                ps,
                                xb_blk[:, kt, j2 * P:(j2 + 1) * P],
                                wv_t[:, kt, :],
                                start=(kt == 0), stop=(kt == KT - 1))
                        nc.vector.tensor_copy(v_sb[:, jt, :], ps)

            # ---------------- phase 2: attention ----------------
            with tc.tile_pool(name="expp", bufs=6) as expp, \
                 tc.tile_pool(name="sump", bufs=2) as sump, \
                 tc.tile_pool(name="drainp", bufs=2) as drainp, \
                 tc.tile_pool(name="dotsp", bufs=2, space="PSUM") as dotsp, \
                 tc.tile_pool(name="avp", bufs=1, space="PSUM") as avp, \
                 tc.tile_pool(name="denp", bufs=1, space="PSUM") as denp:
                LAG = 2   # PV/ones trail dots/exp by 2 j-tiles so the PE
                          # never waits on the ACT exp of the current tile
                # prefetch the output-projection weights; DMA is idle here
                nc.sync.dma_start(
                    out=wo_sb, in_=woT.rearrange("(dt p) e -> p dt e", p=P))
                # Denominator: j-tiles 0..11 are pre-summed in quads on the
                # DVE (3 adds each) so one ones-matmul covers 4 tiles; tiles
                # 12..15 are pre-summed in pairs (1 add) so the block tail
                # doesn't serialize behind a deep add tree.  The quad
                # ones-matmuls are scheduled at jt 8/11/14 -- late enough
                # that the previous block's drain chain (po-copy, pd-copy,
                # reciprocal, broadcast, multiply) and this block's adds have
                # cleared the DVE queue before the PE reaches them.  bf16
                # partial sums add ~0.15% rms to the denominator.
                for ib in range(IB2):
                    for h in range(HH):
                        i0 = ib * 1024
                        po = avp.tile([P, 1024], f32)
                        pd = denp.tile([1, 1024], f32)
                        ets = {}
                        etqs = {}
                        for jt in range(NJT + LAG):
                            if jt < NJT:
                                ps = dotsp.tile([P, 1024], f32, tag="ps")
                                k_l = kT_sb[:, h, jt * P:(jt + 1) * P]
                                for hf in range(2):
                                    nc.tensor.matmul(
                                        ps[:, hf * 512:(hf + 1) * 512],
                                        k_l,
                                        qT_sb[:, h,
                                              i0 + hf * 512:i0 + (hf + 1) * 512],
                                        start=True, stop=True)
                                et = expp.tile([P, 1024], bf16, tag="exp")
                                nc.scalar.activation(et, ps, Exp,
                                                     scale=SCALE_EXP)
                                ets[jt] = et
                            if jt >= LAG:
                                jd = jt - LAG
                                et = ets[jd]
                                v_l = v_sb[:, jd, h * P:(h + 1) * P]
                                for hf in range(2):
                                    sl = slice(hf * 512, (hf + 1) * 512)
                                    nc.tensor.matmul(
                                        po[:, sl], v_l, et[:, sl],
                                        start=(jd == 0), stop=(jd == NJT - 1))
                            if jt in (3, 7, 11):
                                q = jt // 4
                                s01 = sump.tile([P, 1024], bf16, tag="s01")
                                nc.vector.tensor_add(
                                    s01, ets[4 * q], ets[4 * q + 1])
                                s23 = sump.tile([P, 1024], bf16, tag="s23")
                                nc.vector.tensor_add(
                                    s23, ets[4 * q + 2], ets[4 * q + 3])
                                etq = sump.tile([P, 1024], bf16, tag="etq")
                                nc.vector.tensor_add(etq, s01, s23)
                                etqs[q] = etq
                            if jt in (15, 17):
                                t0 = 12 if jt == 15 else 14
                                ep = sump.tile([P, 1024], bf16, tag="ep")
                                nc.vector.tensor_add(
                                    ep, ets[t0], ets[t0 + 1])
                                etqs[("p", t0)] = ep
                            if jt in (8, 11, 14):
                                q = {8: 0, 11: 1, 14: 2}[jt]
                                etq = etqs.pop(q)
                                for hf in range(2):
                                    sl = slice(hf * 512, (hf + 1) * 512)
                                    nc.tensor.matmul(
                                        pd[:, sl], ones_sb, etq[:, sl],
                                        start=(q == 0), stop=False)
                            if jt in (16, 17):
                                t0 = 12 if jt == 16 else 14
                                ep = etqs.pop(("p", t0))
                                for hf in range(2):
                                    sl = slice(hf * 512, (hf + 1) * 512)
                                    nc.tensor.matmul(
                                        pd[:, sl], ones_sb, ep[:, sl],
                                        start=False, stop=(t0 == 14))
                        # Drain: free the PV and denominator PSUM banks as
                        # fast as possible (next block's accumulations wait
                        # on them), then normalize off the critical path.
                        osl = oT_ts[h][ib]
                        nc.vector.tensor_copy(osl, po)
                        dsb = drainp.tile([1, 1024], f32, tag="den")
                        # on DVE, not ACT: the ACT queue must flow straight
                        # from this block's exp(15) into the next block's
                        # exp(0) or the PE stalls on the dots pipeline refill
                        nc.vector.tensor_copy(dsb, pd)
                        rcf = drainp.tile([1, 1024], f32, tag="rcf")
                        nc.vector.reciprocal_approx_fast(rcf, dsb)
                        rcb = drainp.tile([1, 1024], bf16, tag="rcb")
                        nc.vector.tensor_copy(rcb, rcf)
                        bc = drainp.tile([P, 1024], bf16, tag="bc")
                        nc.gpsimd.partition_broadcast(bc, rcb)
                        nc.vector.tensor_mul(osl, osl, bc)

                # ---------------- phase 3: output projection ----------------
                # y-psum tiles share the dots pool slots (tag "ps"), which
                # free as the exp of the final j-tiles completes -- a fresh
                # PSUM pool would wait on the whole attention stack instead.
                with tc.tile_pool(name="ystage", bufs=4) as ystage:
                    for ib in range(NIB):
                        bb, half = divmod(ib, 2)
                        for e8 in range(DIM // P):
                            ps = dotsp.tile([P, 512], f32, tag="ps")
                            for dt in range(HH):
                                nc.tensor.matmul(
                                    ps,
                                    wo_sb[:, dt, e8 * P:(e8 + 1) * P],
                                    oT_ts[dt][bb][:, half * 512:(half + 1) * 512],
                                    start=(dt == 0), stop=(dt == HH - 1))
                            ys = ystage.tile([P, 512], bf16, tag="ys")
                            if ib % 2 == 0:
                                nc.vector.tensor_copy(ys, ps)
                            else:
                                nc.scalar.copy(ys, ps)
                            nc.sync.dma_start(
                                out=yT[e8 * P:(e8 + 1) * P,
                                       ib * 512:(ib + 1) * 512],
                                in_=ys)

    nc.compile()
    return nc


_nc_by_reps = {}


def _get_nc(reps=1):
    if reps not in _nc_by_reps:
        _nc_by_reps[reps] = _build_nc(reps)
    return _nc_by_reps[reps]


def _make_in_maps(x_a, x_b, W_q, W_kv, W_out):
    from concourse import mybir
    BF = mybir.dt.np(mybir.dt.bfloat16)
    xaT = [np.ascontiguousarray(x_a[b].T).astype(BF) for b in range(B)]
    xbT = [np.ascontiguousarray(x_b[b].T).astype(BF) for b in range(B)]
    in_maps = []
    for c in range(NCORES):
        b, hh = divmod(c, 2)
        hs = hh * DVC
        in_maps.append({
            "xaT": xaT[b],
            "xbT": xbT[b],
            "wqT": np.ascontiguousarray(W_q[hs:hs + DVC].T).astype(BF),
            "wkT": np.ascontiguousarray(W_kv[hs:hs + DVC].T).astype(BF),
            "wvT": np.ascontiguousarray(
                W_kv[DIM + hs:DIM + hs + DVC].T).astype(BF),
            "woT": np.ascontiguousarray(W_out[:, hs:hs + DVC].T).astype(BF),
            "ones": np.ones((P, 1), dtype=BF),
        })
    return in_maps


def kernel(x_a, x_b, W_q, W_kv, W_out, b_out):
    global LAST_EXEC_NS, LAST_RESULTS
    from concourse import bass_utils

    x_a = np.asarray(x_a, dtype=np.float32)
    x_b = np.asarray(x_b, dtype=np.float32)
    W_q = np.asarray(W_q, dtype=np.float32)
    W_kv = np.asarray(W_kv, dtype=np.float32)
    W_out = np.asarray(W_out, dtype=np.float32)
    b_out = np.asarray(b_out, dtype=np.float32)

    nc = _get_nc(REPS)
    in_maps = _make_in_maps(x_a, x_b, W_q, W_kv, W_out)

    res = bass_utils.run_bass_kernel_spmd(
        nc, in_maps, core_ids=list(range(NCORES)), trace=_TRACE)
    LAST_EXEC_NS = res.exec_time_ns
    LAST_RESULTS = res

    out = np.empty((B, N, DIM), dtype=np.float32)
    for b in range(B):
        acc = (res.results[2 * b]["yT"].astype(np.float32)
               + res.results[2 * b + 1]["yT"].astype(np.float32))
        out[b] = acc.T + b_out
    return out


def bench(inputs, reps_pair=(1, 9), iters=5):
    """Measure on-device time per kernel body via rep-delta wall timing."""
    import time
    from concourse import bass_utils
    ins = {k: np.asarray(v, dtype=np.float32) for k, v in inputs.items()
           if k != "b_out"}
    in_maps = _make_in_maps(ins["x_a"], ins["x_b"], ins["W_q"], ins["W_kv"],
                            ins["W_out"])
    walls = {}
    for reps in reps_pair:
        nc = _get_nc(reps)
        # warm-up (compile+cache)
        bass_utils.run_bass_kernel_spmd(nc, in_maps, core_ids=list(range(NCORES)))
        ts = []
        for _ in range(iters):
            t0 = time.perf_counter()
            bass_utils.run_bass_kernel_spmd(nc, in_maps,
                                            core_ids=list(range(NCORES)))
            ts.append(time.perf_counter() - t0)
        walls[reps] = min(ts)
        print(f"reps={reps}: wall min={walls[reps]*1e3:.2f} ms  all={[f'{t*1e3:.1f}' for t in ts]}")
    r0, r1 = reps_pair
    ns = (walls[r1] - walls[r0]) / (r1 - r0) * 1e9
    print(f"per-body device time: {ns:.0f} ns")
    return ns
